# revision 32
# baseline (speedup 1.0000x reference)
"""Distributed causal multi-head attention (QKV projection + flash attention)
for Trainium2, sharded head-parallel across 8 NeuronCores.

Problem: x[2,2048,1024] @ W[1024,3072] + b -> qkv; causal softmax attention
(16 heads, head_dim 64); output [2,2048,16,64].

Sharding: core c handles batch c//4 and the 4 heads 4*(c%4)..4*(c%4)+3.
Each core's output slice is disjoint -> no collectives.

Device kernel (per core, bf16 matmuls with fp32 PSUM accumulation):
  - host passes x pre-transposed (xT [1024,2048] bf16) and W column-sliced,
    reordered and bf16-converted
  - projection: qT/kT produced transposed ([head-pair 128, S]) with W as the
    stationary operand; v produced natural ([S,64] tiles) with xT stationary
  - attention per head-pair: scoresT[sk,sq] = kT.T @ qT row-packed 2 heads per
    PE pass (K=64 each, tile_position rows 0-63 / 64-127) into one 2-bank PSUM
    tile; one fused exp per block on ACT (scale=1/8); causal via partial-width
    blocks + a [128,128] triangular additive mask on diagonal windows; PV
    accumulates outT[65, sq] += v'[sk,65].T @ expT[sk,sq] where v' has a ones
    column -> row 64 = softmax denominator.
  - scheduling: projection matmuls are emitted at LATE Tile priority so the
    greedy scheduler uses them as PE filler whenever attention stalls on ACT
    (exp); xt loads go over HWDGE so the Pool engine stays free.
  - output: unnormalized [4, 65, 2048] f32; host divides by row 64, adds the
    v bias, transposes into the full output.
"""

import numpy as np
from contextlib import contextmanager

NUM_HEAD = 16
HEAD_DIM = 64
HIDDEN = 1024
B, S = 2, 2048
N_CORES = 8
HPC = 4          # heads per core
NCH = 4          # sq chunks of 512
CHW = 512        # chunk width
NT = 16          # sk tiles of 128
KB = 8           # k-dim blocks of 128
NEG = -1.0e9
SCALE = HEAD_DIM ** -0.5

_CACHE = {}


def _build(repeat=1):
    import concourse.bacc as bacc
    import concourse.mybir as mybir
    import concourse.tile as tile

    f32 = mybir.dt.float32
    bf16 = mybir.dt.bfloat16
    AF = mybir.ActivationFunctionType

    nc = bacc.Bacc("TRN2", target_bir_lowering=False, debug=False)

    # host pre-shuffles all inputs into SBUF-partition-major layouts so each
    # load is one large DMA with 1-8KB contiguous runs per partition
    XT = nc.dram_tensor("XT", [128, NCH, KB, CHW], bf16, kind="ExternalInput")
    WQK = nc.dram_tensor("WQK", [128, KB, 512], bf16, kind="ExternalInput")
    WV = nc.dram_tensor("WV", [128, KB, 256], bf16, kind="ExternalInput")
    BQKT = nc.dram_tensor("BQKT", [128, 4], f32, kind="ExternalInput")
    TRI = nc.dram_tensor("TRI", [128, 2, 128], f32, kind="ExternalInput")
    OUT = nc.dram_tensor("OUT", [HPC, 65, S], f32, kind="ExternalOutput")

    with tile.TileContext(nc) as tc:

        @contextmanager
        def prio(v):
            saved = tc.cur_priority
            tc.cur_priority = v
            try:
                yield
            finally:
                tc.cur_priority = saved

        with tc.tile_pool(name="const", bufs=1) as const_pool, \
             tc.tile_pool(name="qkv", bufs=1) as qkv_pool, \
             tc.tile_pool(name="xt", bufs=4) as xt_pool, \
             tc.tile_pool(name="exps", bufs=12) as exp_pool, \
             tc.tile_pool(name="outs", bufs=4) as out_pool, \
             tc.tile_pool(name="ps_sc", bufs=2, space="PSUM") as ps_sc, \
             tc.tile_pool(name="ps_pr", bufs=2, space="PSUM") as ps_pr, \
             tc.tile_pool(name="ps_pv", bufs=2, space="PSUM") as ps_pv:

            for _rep in range(repeat):
                PROJ_PRIO = 1_000_000 * (_rep + 1)

                wqk_sb = const_pool.tile([128, KB, 512], bf16, tag="wqk")
                wv_sb = const_pool.tile([128, KB, 256], bf16, tag="wv")
                bqk_sb = const_pool.tile([128, 4], f32, tag="bqk")
                tri_sb = const_pool.tile([128, 2, 128], f32, tag="tri")
                warm = const_pool.tile([128, 1], f32, tag="warm")

                nc.sync.dma_start(wqk_sb[:, :, 0:256], WQK[:, :, 0:256])
                nc.sync.dma_start(bqk_sb[:], BQKT[:])
                nc.sync.dma_start(tri_sb[:], TRI[:])
                nc.sync.dma_start(wqk_sb[:, :, 256:512], WQK[:, :, 256:512])
                nc.sync.dma_start(wv_sb[:], WV[:])
                # hoist the ACT exp table load off the critical path
                nc.scalar.activation(warm[:], tri_sb[:, 0, 0:1], AF.Exp, scale=SCALE)
                # low-priority dummy matmuls: warm the PE clock (HAM) during
                # the initial weight/x DMA wait; greedy scheduler only runs
                # them when no real matmul is ready
                dummy_in = const_pool.tile([128, 640], bf16, tag="dummy")
                nc.vector.memset(dummy_in[:], 0.0)
                with prio(3_000_000):
                    for _ in range(20):
                        psd = ps_pr.tile([128, CHW], f32, tag="pr")
                        nc.tensor.matmul(psd[:], dummy_in[:, 0:128],
                                         dummy_in[:, 128:640],
                                         start=True, stop=True)

                # qT2/kT2: [pair, 128 (2 heads x 64 d), S]; v: [sk-tile, head, 65]
                qT2 = qkv_pool.tile([128, 2, S], bf16, tag="qT2")
                kT2 = qkv_pool.tile([128, 2, S], bf16, tag="kT2")
                v_sb = qkv_pool.tile([128, NT, HPC, 65], bf16, tag="v")
                nc.vector.memset(v_sb[:, :, :, 64], 1.0)

                # chunk 0 in two halves so its projection starts sooner;
                # later chunks as one whole-chunk DMA each (SWDGE, Pool)
                xts = []
                for C in range(NCH):
                    xt = xt_pool.tile([128, KB, CHW], bf16, tag="xt")
                    if C == 0:
                        nc.gpsimd.dma_start(xt[:, 0:4, :], XT[:, 0, 0:4, :])
                        nc.gpsimd.dma_start(xt[:, 4:8, :], XT[:, 0, 4:8, :])
                    else:
                        nc.gpsimd.dma_start(xt[:, :, :], XT[:, C])
                    xts.append(xt)

                def emit_qkT_group(C, blk):
                    # col-blocks: [q_p0, k_p0, q_p1, k_p1] so pair 0's
                    # weights are the first contiguous half of WQK
                    xt = xts[C]
                    ps = ps_pr.tile([128, CHW], f32, tag="pr")
                    for kb in range(KB):
                        nc.tensor.matmul(
                            ps[:],
                            wqk_sb[:, kb, blk * 128:(blk + 1) * 128],
                            xt[:, kb, :],
                            start=(kb == 0), stop=(kb == KB - 1))
                    dest = (qT2 if blk % 2 == 0 else kT2)[:, blk // 2,
                                                          C * CHW:(C + 1) * CHW]
                    nc.vector.tensor_scalar_add(dest, ps[:],
                                                bqk_sb[:, blk:blk + 1])

                def emit_v_group(C, rt):
                    xt = xts[C]
                    t = C * 4 + rt
                    psv = ps_pr.tile([128, 256], f32, tag="pr")
                    for kb in range(KB):
                        nc.tensor.matmul(
                            psv[:],
                            xt[:, kb, rt * 128:(rt + 1) * 128],
                            wv_sb[:, kb, :],
                            start=(kb == 0), stop=(kb == KB - 1))
                    nc.vector.tensor_copy(v_sb[:, t, :, 0:64], psv[:])

                def proj_pair(C, p):
                    # groups needed by pair p's attention: q blk p, k blk 2+p,
                    # plus (for p==0) all v tiles of this chunk
                    emit_qkT_group(C, 2 * p)
                    emit_qkT_group(C, 2 * p + 1)
                    if p == 0:
                        for rt in range(4):
                            emit_v_group(C, rt)

                def attn_pair(C, p):
                    hA, hB = 2 * p, 2 * p + 1
                    nblk = 4 * C + 4
                    pvA = ps_pv.tile([128, CHW], f32, tag="pv")
                    pvB = ps_pv.tile([128, CHW], f32, tag="pv")
                    for i in range(nblk):
                        m = i - 4 * C
                        off = 0 if m < 0 else 128 * m
                        w = CHW - off
                        sqs = C * CHW + off
                        psM = ps_sc.tile([128, 2, CHW], f32, tag="sc")
                        nc.tensor.matmul(
                            psM[:, 0, 0:w],
                            kT2[0:64, p, i * 128:(i + 1) * 128],
                            qT2[0:64, p, sqs:sqs + w],
                            start=True, stop=True, tile_position=(0, 0))
                        nc.tensor.matmul(
                            psM[:, 1, 0:w],
                            kT2[64:128, p, i * 128:(i + 1) * 128],
                            qT2[64:128, p, sqs:sqs + w],
                            start=True, stop=True, tile_position=(64, 0))
                        expM = exp_pool.tile([128, 2, CHW], bf16, tag="exp")
                        if m >= 0:
                            nc.vector.tensor_add(psM[:, :, 0:128],
                                                 psM[:, :, 0:128], tri_sb[:])
                        # one exp per block over both heads (strided AP)
                        nc.scalar.activation(expM[:, :, 0:w], psM[:, :, 0:w],
                                             AF.Exp, scale=SCALE)
                        nc.tensor.matmul(
                            pvA[0:65, off:CHW], v_sb[:, i, hA, :],
                            expM[:, 0, 0:w],
                            start=(i == 0), stop=(i == nblk - 1))
                        nc.tensor.matmul(
                            pvB[0:65, off:CHW], v_sb[:, i, hB, :],
                            expM[:, 1, 0:w],
                            start=(i == 0), stop=(i == nblk - 1))
                    oA = out_pool.tile([128, CHW], f32, tag="o")
                    oB = out_pool.tile([128, CHW], f32, tag="o")
                    nc.vector.tensor_copy(oA[0:65, :], pvA[0:65, :])
                    nc.vector.tensor_copy(oB[0:65, :], pvB[0:65, :])
                    nc.sync.dma_start(OUT[hA, :, C * CHW:(C + 1) * CHW],
                                      oA[0:65, :])
                    nc.sync.dma_start(OUT[hB, :, C * CHW:(C + 1) * CHW],
                                      oB[0:65, :])

                for C in range(NCH):
                    with prio(PROJ_PRIO + tc.cur_priority):
                        proj_pair(C, 0)
                        proj_pair(C, 1)
                    attn_pair(C, 0)
                    attn_pair(C, 1)

    nc.compile()
    return nc


def _get_nc(repeat=1):
    key = ("nc", repeat)
    if key not in _CACHE:
        _CACHE[key] = _build(repeat)
    return _CACHE[key]


def _prep_inputs(x, W, b):
    import ml_dtypes
    bf16 = ml_dtypes.bfloat16

    x = np.asarray(x, dtype=np.float32)
    W = np.asarray(W, dtype=np.float32)
    b = np.asarray(b, dtype=np.float32)

    W4 = W.reshape(HIDDEN, 3, NUM_HEAD, HEAD_DIM)
    b4 = b.reshape(3, NUM_HEAD, HEAD_DIM)

    # [p, chunk, kb, col] layout: per partition, each chunk is an 8KB run
    xT = [np.ascontiguousarray(
              x[bi].T.reshape(KB, 128, NCH, CHW).transpose(1, 2, 0, 3)
          ).astype(bf16) for bi in range(B)]

    tri = np.where(np.arange(128)[None, :] >= np.arange(128)[:, None],
                   np.float32(0.0), np.float32(NEG)).astype(np.float32)
    tri = np.ascontiguousarray(np.stack([tri, tri], axis=1))  # [128, 2, 128]

    in_maps = []
    for c in range(N_CORES):
        bi, g = divmod(c, HPC)
        heads = [4 * g + j for j in range(HPC)]
        wqk = np.concatenate(
            [W4[:, 0, heads[0]], W4[:, 0, heads[1]],
             W4[:, 1, heads[0]], W4[:, 1, heads[1]],
             W4[:, 0, heads[2]], W4[:, 0, heads[3]],
             W4[:, 1, heads[2]], W4[:, 1, heads[3]]],
            axis=1)  # [1024, 512] cols = [q_p0, k_p0, q_p1, k_p1]
        wv = np.concatenate([W4[:, 2, h, :] for h in heads], axis=1)  # [1024,256]
        bqkt = np.stack(
            [np.concatenate([b4[0, heads[0]], b4[0, heads[1]]]),
             np.concatenate([b4[1, heads[0]], b4[1, heads[1]]]),
             np.concatenate([b4[0, heads[2]], b4[0, heads[3]]]),
             np.concatenate([b4[1, heads[2]], b4[1, heads[3]]])],
            axis=1)  # [128, 4] cols = [bq_p0, bk_p0, bq_p1, bk_p1]
        in_maps.append({
            "XT": xT[bi],
            "WQK": np.ascontiguousarray(
                wqk.reshape(KB, 128, 512).transpose(1, 0, 2)).astype(bf16),
            "WV": np.ascontiguousarray(
                wv.reshape(KB, 128, 256).transpose(1, 0, 2)).astype(bf16),
            "BQKT": np.ascontiguousarray(bqkt),
            "TRI": tri,
        })
    return in_maps, b4


def kernel(x, W, b):
    from concourse.bass_utils import run_bass_kernel_spmd

    in_maps, b4 = _prep_inputs(x, W, b)
    nc = _get_nc()
    res = run_bass_kernel_spmd(nc, in_maps, core_ids=list(range(N_CORES)))

    out = np.empty((B, S, NUM_HEAD, HEAD_DIM), dtype=np.float32)
    for c in range(N_CORES):
        bi, g = divmod(c, HPC)
        u = res.results[c]["OUT"]               # [4, 65, 2048]
        o = u[:, :64, :] / u[:, 64:65, :]        # [4, 64, 2048]
        out[bi, :, 4 * g:4 * g + 4, :] = o.transpose(2, 0, 1)
    out += b4[2].reshape(1, 1, NUM_HEAD, HEAD_DIM)
    return out


# revision 33
# speedup vs baseline: 1.6114x; 1.6114x over previous
"""Distributed causal multi-head attention (QKV projection + flash attention)
for Trainium2, sharded head-parallel across 8 NeuronCores.

Problem: x[2,2048,1024] @ W[1024,3072] + b -> qkv; causal softmax attention
(16 heads, head_dim 64); output [2,2048,16,64].

Sharding: core c handles batch c//4 and the 4 heads 4*(c%4)..4*(c%4)+3.
Each core's output slice is disjoint -> no collectives.

Device kernel (per core, bf16 matmuls with fp32 PSUM accumulation):
  - host passes x pre-transposed (xT [1024,2048] bf16) and W column-sliced,
    reordered and bf16-converted
  - projection: qT/kT produced transposed ([head-pair 128, S]) with W as the
    stationary operand; v produced natural ([S,64] tiles) with xT stationary
  - attention per head-pair: scoresT[sk,sq] = kT.T @ qT row-packed 2 heads per
    PE pass (K=64 each, tile_position rows 0-63 / 64-127) into one 2-bank PSUM
    tile; one fused exp per block on ACT (scale=1/8); causal via partial-width
    blocks + a [128,128] triangular additive mask on diagonal windows; PV
    accumulates outT[65, sq] += v'[sk,65].T @ expT[sk,sq] where v' has a ones
    column -> row 64 = softmax denominator.
  - scheduling: projection matmuls are emitted at LATE Tile priority so the
    greedy scheduler uses them as PE filler whenever attention stalls on ACT
    (exp); xt loads go over HWDGE so the Pool engine stays free.
  - output: unnormalized [4, 65, 2048] f32; host divides by row 64, adds the
    v bias, transposes into the full output.
"""

import numpy as np
from contextlib import contextmanager

NUM_HEAD = 16
HEAD_DIM = 64
HIDDEN = 1024
B, S = 2, 2048
N_CORES = 8
HPC = 4          # heads per core
NCH = 4          # sq chunks of 512
CHW = 512        # chunk width
NT = 16          # sk tiles of 128
KB = 8           # k-dim blocks of 128
NEG = -1.0e9
SCALE = HEAD_DIM ** -0.5

_CACHE = {}


def _build(repeat=1):
    import concourse.bacc as bacc
    import concourse.mybir as mybir
    import concourse.tile as tile

    f32 = mybir.dt.float32
    bf16 = mybir.dt.bfloat16
    AF = mybir.ActivationFunctionType

    nc = bacc.Bacc("TRN2", target_bir_lowering=False, debug=False)

    # host pre-shuffles all inputs into SBUF-partition-major layouts so each
    # load is one large DMA with 1-8KB contiguous runs per partition
    XT = nc.dram_tensor("XT", [128, NCH, KB, CHW], bf16, kind="ExternalInput")
    WQK = nc.dram_tensor("WQK", [128, KB, 512], bf16, kind="ExternalInput")
    WV = nc.dram_tensor("WV", [128, KB, 256], bf16, kind="ExternalInput")
    BQKT = nc.dram_tensor("BQKT", [128, 4], f32, kind="ExternalInput")
    TRI = nc.dram_tensor("TRI", [128, 128], f32, kind="ExternalInput")
    OUT = nc.dram_tensor("OUT", [HPC, 65, S], f32, kind="ExternalOutput")

    with tile.TileContext(nc) as tc:

        @contextmanager
        def prio(v):
            saved = tc.cur_priority
            tc.cur_priority = v
            try:
                yield
            finally:
                tc.cur_priority = saved

        with tc.tile_pool(name="const", bufs=1) as const_pool, \
             tc.tile_pool(name="qkv", bufs=1) as qkv_pool, \
             tc.tile_pool(name="xt", bufs=4) as xt_pool, \
             tc.tile_pool(name="exps", bufs=12) as exp_pool, \
             tc.tile_pool(name="outs", bufs=4) as out_pool, \
             tc.tile_pool(name="ps_sc", bufs=2, space="PSUM") as ps_sc, \
             tc.tile_pool(name="ps_pr", bufs=2, space="PSUM") as ps_pr, \
             tc.tile_pool(name="ps_pv", bufs=2, space="PSUM") as ps_pv:

            for _rep in range(repeat):
                PROJ_PRIO = 1_000_000 * (_rep + 1)

                wqk_sb = const_pool.tile([128, KB, 512], bf16, tag="wqk")
                wv_sb = const_pool.tile([128, KB, 256], bf16, tag="wv")
                bqk_sb = const_pool.tile([128, 4], f32, tag="bqk")
                tri_sb = const_pool.tile([128, 128], f32, tag="tri")
                warm = const_pool.tile([128, 1], f32, tag="warm")

                nc.sync.dma_start(wqk_sb[:, :, 0:256], WQK[:, :, 0:256])
                nc.sync.dma_start(bqk_sb[:], BQKT[:])
                nc.sync.dma_start(tri_sb[:], TRI[:])
                nc.sync.dma_start(wqk_sb[:, :, 256:512], WQK[:, :, 256:512])
                nc.sync.dma_start(wv_sb[:], WV[:])
                # hoist the ACT exp table load off the critical path
                nc.scalar.activation(warm[:], tri_sb[:, 0:1], AF.Exp, scale=SCALE)
                # low-priority dummy matmuls: warm the PE clock (HAM) during
                # the initial weight/x DMA wait; greedy scheduler only runs
                # them when no real matmul is ready
                dummy_in = const_pool.tile([128, 640], bf16, tag="dummy")
                nc.vector.memset(dummy_in[:], 0.0)
                with prio(3_000_000):
                    for _ in range(20):
                        psd = ps_pr.tile([128, CHW], f32, tag="pr")
                        nc.tensor.matmul(psd[:], dummy_in[:, 0:128],
                                         dummy_in[:, 128:640],
                                         start=True, stop=True)

                # qT2/kT2: [pair, 128 (2 heads x 64 d), S]; v: [sk-tile, head, 65]
                qT2 = qkv_pool.tile([128, 2, S], bf16, tag="qT2")
                kT2 = qkv_pool.tile([128, 2, S], bf16, tag="kT2")
                v_sb = qkv_pool.tile([128, NT, HPC, 65], bf16, tag="v")
                nc.vector.memset(v_sb[:, :, :, 64], 1.0)

                # chunk 0 in two halves so its projection starts sooner;
                # later chunks as one whole-chunk DMA each (SWDGE, Pool)
                xts = []
                for C in range(NCH):
                    xt = xt_pool.tile([128, KB, CHW], bf16, tag="xt")
                    if C == 0:
                        nc.gpsimd.dma_start(xt[:, 0:4, :], XT[:, 0, 0:4, :])
                        nc.gpsimd.dma_start(xt[:, 4:8, :], XT[:, 0, 4:8, :])
                    else:
                        nc.gpsimd.dma_start(xt[:, :, :], XT[:, C])
                    xts.append(xt)

                def emit_qkT_group(C, blk):
                    # col-blocks: [q_p0, k_p0, q_p1, k_p1] so pair 0's
                    # weights are the first contiguous half of WQK
                    xt = xts[C]
                    ps = ps_pr.tile([128, CHW], f32, tag="pr")
                    for kb in range(KB):
                        nc.tensor.matmul(
                            ps[:],
                            wqk_sb[:, kb, blk * 128:(blk + 1) * 128],
                            xt[:, kb, :],
                            start=(kb == 0), stop=(kb == KB - 1))
                    dest = (qT2 if blk % 2 == 0 else kT2)[:, blk // 2,
                                                          C * CHW:(C + 1) * CHW]
                    nc.vector.tensor_scalar_add(dest, ps[:],
                                                bqk_sb[:, blk:blk + 1])

                def emit_v_group(C, rt):
                    xt = xts[C]
                    t = C * 4 + rt
                    psv = ps_pr.tile([128, 256], f32, tag="pr")
                    for kb in range(KB):
                        nc.tensor.matmul(
                            psv[:],
                            xt[:, kb, rt * 128:(rt + 1) * 128],
                            wv_sb[:, kb, :],
                            start=(kb == 0), stop=(kb == KB - 1))
                    nc.vector.tensor_copy(v_sb[:, t, :, 0:64], psv[:])

                def proj_pair(C, p):
                    # groups needed by pair p's attention: q blk p, k blk 2+p,
                    # plus (for p==0) all v tiles of this chunk
                    emit_qkT_group(C, 2 * p)
                    emit_qkT_group(C, 2 * p + 1)
                    if p == 0:
                        for rt in range(4):
                            emit_v_group(C, rt)

                def attn_pair(C, p):
                    hA, hB = 2 * p, 2 * p + 1
                    nblk = 4 * C + 4
                    pvA = ps_pv.tile([128, CHW], f32, tag="pv")
                    pvB = ps_pv.tile([128, CHW], f32, tag="pv")
                    for i in range(nblk):
                        m = i - 4 * C
                        off = 0 if m < 0 else 128 * m
                        w = CHW - off
                        sqs = C * CHW + off
                        psM = ps_sc.tile([128, 2 * CHW], f32, tag="sc")
                        nc.tensor.matmul(
                            psM[:, 0:w],
                            kT2[0:64, p, i * 128:(i + 1) * 128],
                            qT2[0:64, p, sqs:sqs + w],
                            start=True, stop=True, tile_position=(0, 0))
                        nc.tensor.matmul(
                            psM[:, CHW:CHW + w],
                            kT2[64:128, p, i * 128:(i + 1) * 128],
                            qT2[64:128, p, sqs:sqs + w],
                            start=True, stop=True, tile_position=(64, 0))
                        expM = exp_pool.tile([128, 2 * CHW], bf16, tag="exp")
                        if m >= 0:
                            nc.vector.tensor_add(psM[:, 0:128], psM[:, 0:128],
                                                 tri_sb[:])
                            nc.vector.tensor_add(psM[:, CHW:CHW + 128],
                                                 psM[:, CHW:CHW + 128],
                                                 tri_sb[:])
                        # one fused exp per block; cols [w:CHW] of partial
                        # blocks hold stale-but-finite psum data and are never
                        # consumed by the PV matmuls below
                        nc.scalar.activation(expM[:, 0:CHW + w],
                                             psM[:, 0:CHW + w],
                                             AF.Exp, scale=SCALE)
                        nc.tensor.matmul(
                            pvA[0:65, off:CHW], v_sb[:, i, hA, :],
                            expM[:, 0:w],
                            start=(i == 0), stop=(i == nblk - 1))
                        nc.tensor.matmul(
                            pvB[0:65, off:CHW], v_sb[:, i, hB, :],
                            expM[:, CHW:CHW + w],
                            start=(i == 0), stop=(i == nblk - 1))
                    oA = out_pool.tile([128, CHW], f32, tag="o")
                    oB = out_pool.tile([128, CHW], f32, tag="o")
                    nc.vector.tensor_copy(oA[0:65, :], pvA[0:65, :])
                    nc.vector.tensor_copy(oB[0:65, :], pvB[0:65, :])
                    nc.sync.dma_start(OUT[hA, :, C * CHW:(C + 1) * CHW],
                                      oA[0:65, :])
                    nc.sync.dma_start(OUT[hB, :, C * CHW:(C + 1) * CHW],
                                      oB[0:65, :])

                for C in range(NCH):
                    with prio(PROJ_PRIO + tc.cur_priority):
                        proj_pair(C, 0)
                        proj_pair(C, 1)
                    attn_pair(C, 0)
                    attn_pair(C, 1)

    nc.compile()
    return nc


def _get_nc(repeat=1):
    key = ("nc", repeat)
    if key not in _CACHE:
        _CACHE[key] = _build(repeat)
    return _CACHE[key]


def _prep_inputs(x, W, b):
    import ml_dtypes
    bf16 = ml_dtypes.bfloat16

    x = np.asarray(x, dtype=np.float32)
    W = np.asarray(W, dtype=np.float32)
    b = np.asarray(b, dtype=np.float32)

    W4 = W.reshape(HIDDEN, 3, NUM_HEAD, HEAD_DIM)
    b4 = b.reshape(3, NUM_HEAD, HEAD_DIM)

    # [p, chunk, kb, col] layout: per partition, each chunk is an 8KB run
    xT = [np.ascontiguousarray(
              x[bi].T.reshape(KB, 128, NCH, CHW).transpose(1, 2, 0, 3)
          ).astype(bf16) for bi in range(B)]

    tri = np.where(np.arange(128)[None, :] >= np.arange(128)[:, None],
                   np.float32(0.0), np.float32(NEG)).astype(np.float32)

    in_maps = []
    for c in range(N_CORES):
        bi, g = divmod(c, HPC)
        heads = [4 * g + j for j in range(HPC)]
        wqk = np.concatenate(
            [W4[:, 0, heads[0]], W4[:, 0, heads[1]],
             W4[:, 1, heads[0]], W4[:, 1, heads[1]],
             W4[:, 0, heads[2]], W4[:, 0, heads[3]],
             W4[:, 1, heads[2]], W4[:, 1, heads[3]]],
            axis=1)  # [1024, 512] cols = [q_p0, k_p0, q_p1, k_p1]
        wv = np.concatenate([W4[:, 2, h, :] for h in heads], axis=1)  # [1024,256]
        bqkt = np.stack(
            [np.concatenate([b4[0, heads[0]], b4[0, heads[1]]]),
             np.concatenate([b4[1, heads[0]], b4[1, heads[1]]]),
             np.concatenate([b4[0, heads[2]], b4[0, heads[3]]]),
             np.concatenate([b4[1, heads[2]], b4[1, heads[3]]])],
            axis=1)  # [128, 4] cols = [bq_p0, bk_p0, bq_p1, bk_p1]
        in_maps.append({
            "XT": xT[bi],
            "WQK": np.ascontiguousarray(
                wqk.reshape(KB, 128, 512).transpose(1, 0, 2)).astype(bf16),
            "WV": np.ascontiguousarray(
                wv.reshape(KB, 128, 256).transpose(1, 0, 2)).astype(bf16),
            "BQKT": np.ascontiguousarray(bqkt),
            "TRI": tri,
        })
    return in_maps, b4


def kernel(x, W, b):
    from concourse.bass_utils import run_bass_kernel_spmd

    in_maps, b4 = _prep_inputs(x, W, b)
    nc = _get_nc()
    res = run_bass_kernel_spmd(nc, in_maps, core_ids=list(range(N_CORES)))

    out = np.empty((B, S, NUM_HEAD, HEAD_DIM), dtype=np.float32)
    for c in range(N_CORES):
        bi, g = divmod(c, HPC)
        u = res.results[c]["OUT"]               # [4, 65, 2048]
        o = u[:, :64, :] / u[:, 64:65, :]        # [4, 64, 2048]
        out[bi, :, 4 * g:4 * g + 4, :] = o.transpose(2, 0, 1)
    out += b4[2].reshape(1, 1, NUM_HEAD, HEAD_DIM)
    return out
